# revision 13
# baseline (speedup 1.0000x reference)
"""TRN2 Bass kernel for DSV3.2-style sparse attention (lightning indexer +
top-k masked SDPA + indexer KL loss), distributed over 8 NeuronCores with
context parallelism on the query axis.

Self-contained: hardcodes shapes b=1, h=hi=16, s=2048, d=di=128, K=512.
kernel(**inputs) takes full unsharded inputs, returns (loss, output) to
match the reference.

Algorithm (per core, 256 query rows = 2 chunks of 128):
  1. Indexer scores isc[q,j] = sum_h relu(w[q,h]*qi_h[q]@ki[j]) via fp32
     matmuls (w folded into qi host-side), relu+accumulate on DVE.
  2. Causal additive sentinel (host-supplied -1e30 mask), per-row top-512
     threshold via 26-step vectorized bisection (count = fused
     tensor_scalar+accum), keep mask m = isc > T. Rows q<512 keep all
     causal entries automatically (lo init -0.5, scores >= 0).
  3. Main attention in transposed layout: S^T[k,q] per 128-key tile
     (bf16), exp on ACT (no row-max: |logits|<~7), mask-multiply by m^T,
     AV matmul with a ones-column appended to V giving Z for free;
     output = psum/Z.
  4. Loss: mad = sum_h exp(S - lnZ_h) accumulated in PSUM via identity
     matmuls; KL terms as masked dense sums (tgt normalizer == 16).
"""
import numpy as np
import ml_dtypes

import concourse.bass as bass
import concourse.mybir as mybir
from concourse.tile import TileContext
from concourse.bass_utils import run_bass_kernel_spmd
from concourse.vector_clock import ScopedClock

F32 = mybir.dt.float32
BF16 = mybir.dt.bfloat16
ALU = mybir.AluOpType
ACTF = mybir.ActivationFunctionType

N_CORES = 8
B, H, S, D = 1, 16, 2048, 128
HI, DI = 16, 128
K_TOP = 512
ROWS = S // N_CORES          # 256 rows per core
NKT = S // 128               # 16 key tiles
SEARCH_ITERS = 26
NEG_BIG = -1.0e30
EPS = 1e-8


# ---------------------------------------------------------------------------
# walrus in this env rejects >2 sem-waits on one Drain: split the Tile tail
# drain's waits onto individual nops.
def _patched_drain_and_barrier(self, tick_clock, wait_clock):
    nc = self.nc
    probe = nc.sync.nop()
    wait_clock.add_sem_waits(probe.ins, ScopedClock({None: tick_clock.global_clock}))
    waits = list(probe.ins.sync_info.on_wait or []) if probe.ins.sync_info else []
    if len(waits) > 1:
        probe.ins.sync_info = mybir.SyncInfo(on_wait=[waits[0]], on_update=[])
        for w in waits[1:]:
            n = nc.sync.nop()
            n.ins.sync_info = mybir.SyncInfo(on_wait=[w], on_update=[])
    nc.sync.drain()
    nc.all_engine_barrier()
    assert self.sems is not None
    popped = nc._tile_sem_poison_stack.pop()
    assert popped is self._sem_poison
    nc.clear_and_free_semaphores(list(self.sems.allocated().values()))
    nc.all_engine_barrier()


def _apply_patch():
    import concourse.tile as tile
    tile.TileContext._drain_and_barrier = _patched_drain_and_barrier


# ---------------------------------------------------------------------------
def build_program():
    nc = bass.Bass("TRN2", debug=False, num_devices=N_CORES)

    qT = nc.declare_dram_parameter("qT", [H, D, ROWS], BF16, isOutput=False)
    qiT = nc.declare_dram_parameter("qiT", [HI, DI, ROWS], F32, isOutput=False)
    kT = nc.declare_dram_parameter("kT", [H, D, S], BF16, isOutput=False)
    vx = nc.declare_dram_parameter("vx", [H, S, 130], BF16, isOutput=False)
    kiT = nc.declare_dram_parameter("kiT", [DI, S], F32, isOutput=False)
    cm = nc.declare_dram_parameter("cm", [ROWS, S], F32, isOutput=False)
    ident = nc.declare_dram_parameter("ident", [128, 128], BF16, isOutput=False)

    o_out = nc.declare_dram_parameter("o", [H, ROWS, D], F32, isOutput=True)
    kl_out = nc.declare_dram_parameter("kl", [ROWS, 1], F32, isOutput=True)

    with TileContext(nc) as tc:
        with tc.tile_pool(name="const", bufs=1) as constp, \
             tc.tile_pool(name="rowbuf", bufs=1) as rowp, \
             tc.tile_pool(name="small", bufs=1) as smallp:

            kiT_sb = constp.tile([128, S], F32, tag="kiT")
            nc.sync.dma_start(out=kiT_sb[:], in_=kiT[:, :])
            id_sb = constp.tile([128, 128], BF16, tag="ident")
            nc.sync.dma_start(out=id_sb[:], in_=ident[:, :])
            qT_sb = constp.tile([128, H * ROWS], BF16, tag="qT")
            nc.sync.dma_start(
                out=qT_sb[:].rearrange("p (h r) -> p h r", h=H),
                in_=qT.ap().rearrange("h d r -> d h r"))
            qiT_sb = constp.tile([128, HI * ROWS], F32, tag="qiT")
            nc.sync.dma_start(
                out=qiT_sb[:].rearrange("p (h r) -> p h r", h=HI),
                in_=qiT.ap().rearrange("h d r -> d h r"))

            # per-chunk persistent row-state
            isc = [rowp.tile([128, S], F32, tag=f"isc{c}", name=f"isc{c}") for c in range(2)]
            m_bf = [rowp.tile([128, S], BF16, tag=f"m{c}", name=f"m{c}") for c in range(2)]
            mT = rowp.tile([128, NKT * ROWS], BF16, tag="mT")  # [k,(kt q)]
            negmx = [smallp.tile([128, 1], F32, tag=f"negmx{c}", name=f"negmx{c}") for c in range(2)]
            epsb = smallp.tile([128, 1], F32, tag="epsb", name="epsb")
            nc.vector.memset(epsb[:], EPS)
            thr = [smallp.tile([128, 1], F32, tag=f"thr{c}", name=f"thr{c}") for c in range(2)]
            zbuf = [smallp.tile([128, H], F32, tag=f"z{c}", name=f"z{c}") for c in range(2)]
            nlnz = [smallp.tile([128, H], F32, tag=f"nlnz{c}", name=f"nlnz{c}") for c in range(2)]

            # ---------------- phase A/B: indexer scores + threshold + mask
            with tc.tile_pool(name="idxpsum", bufs=2, space="PSUM") as pp, \
                 tc.tile_pool(name="searchtmp", bufs=2) as stp:
                for cc in range(2):
                    qoff = 128 * cc
                    for h in range(HI):
                        lhs = qiT_sb[:, h * ROWS + qoff: h * ROWS + qoff + 128]
                        for kh in range(2):
                            ps = pp.tile([128, S // 2], F32, tag="p",
                                         name="p")
                            for ks in range(2):
                                co = 1024 * kh + 512 * ks
                                nc.tensor.matmul(
                                    ps[:, 512 * ks: 512 * (ks + 1)],
                                    lhsT=lhs,
                                    rhs=kiT_sb[:, co: co + 512],
                                    start=True, stop=True)
                            half = isc[cc][:, 1024 * kh: 1024 * (kh + 1)]
                            if h == 0:
                                nc.vector.tensor_scalar_max(
                                    out=half, in0=ps[:], scalar1=0.0)
                            else:
                                nc.vector.scalar_tensor_tensor(
                                    out=half, in0=ps[:], scalar=0.0,
                                    in1=half, op0=ALU.max, op1=ALU.add)

                    # causal sentinel
                    cm_sb = stp.tile([128, S], F32, tag="cm")
                    nc.sync.dma_start(out=cm_sb[:], in_=cm[qoff:qoff + 128, :])
                    nc.vector.tensor_tensor(
                        out=isc[cc][:], in0=isc[cc][:], in1=cm_sb[:],
                        op=ALU.add)

                    # row max (for pdist later) and search bounds
                    mx = smallp.tile([128, 1], F32, tag="mx")
                    nc.vector.reduce_max(mx[:], isc[cc][:],
                                         axis=mybir.AxisListType.X)
                    nc.vector.tensor_scalar_mul(
                        out=negmx[cc][:], in0=mx[:], scalar1=-1.0)
                    lo = smallp.tile([128, 1], F32, tag="lo")
                    hi = smallp.tile([128, 1], F32, tag="hi")
                    nc.vector.memset(lo[:], -0.5)
                    nc.vector.tensor_scalar_add(out=hi[:], in0=mx[:],
                                                scalar1=1.0)
                    mid = smallp.tile([128, 1], F32, tag="mid")
                    cnt = smallp.tile([128, 1], F32, tag="cnt")
                    cond = smallp.tile([128, 1], mybir.dt.uint8, tag="cond")
                    ncond = smallp.tile([128, 1], mybir.dt.uint8, tag="ncond")
                    junk = stp.tile([128, S], BF16, tag="junk")
                    for _ in range(SEARCH_ITERS):
                        nc.vector.tensor_tensor(out=mid[:], in0=lo[:],
                                                in1=hi[:], op=ALU.add)
                        nc.vector.tensor_scalar_mul(out=mid[:], in0=mid[:],
                                                    scalar1=0.5)
                        nc.vector.tensor_scalar(
                            out=junk[:], in0=isc[cc][:], scalar1=mid[:],
                            scalar2=None, op0=ALU.is_gt, op1=ALU.add,
                            accum_out=cnt[:])
                        nc.vector.tensor_scalar(
                            out=cond[:], in0=cnt[:], scalar1=float(K_TOP),
                            scalar2=None, op0=ALU.is_ge)
                        nc.vector.tensor_scalar(
                            out=ncond[:], in0=cnt[:], scalar1=float(K_TOP),
                            scalar2=None, op0=ALU.is_lt)
                        nc.vector.copy_predicated(lo[:], cond[:], mid[:])
                        nc.vector.copy_predicated(hi[:], ncond[:], mid[:])
                    nc.vector.tensor_copy(thr[cc][:], lo[:])

                    # keep mask (bf16) and its transpose tiles
                    nc.vector.tensor_scalar(
                        out=m_bf[cc][:], in0=isc[cc][:], scalar1=thr[cc][:],
                        scalar2=None, op0=ALU.is_gt)

            with tc.tile_pool(name="tpsum", bufs=2, space="PSUM") as tp:
                for cc in range(2):
                    qoff = 128 * cc
                    for kt in range(NKT):
                        tps = tp.tile([128, 128], BF16, tag="mt", name="mt")
                        nc.tensor.transpose(
                            tps[:], m_bf[cc][:, 128 * kt:128 * (kt + 1)],
                            id_sb[:])
                        nc.vector.tensor_copy(
                            mT[:, ROWS * kt + qoff: ROWS * kt + qoff + 128],
                            tps[:])

            # ---------------- phase C: main attention, head-by-head
            with tc.tile_pool(name="khead", bufs=2) as kp, \
                 tc.tile_pool(name="pt", bufs=2) as ptp, \
                 tc.tile_pool(name="spsum", bufs=2, space="PSUM") as sp, \
                 tc.tile_pool(name="avpsum", bufs=2, space="PSUM") as avp, \
                 tc.tile_pool(name="outsb", bufs=2) as osb:
                for h in range(H):
                    kT_h = kp.tile([128, S], BF16, tag="kT")
                    nc.sync.dma_start(out=kT_h[:], in_=kT[h, :, :])
                    vx_h = kp.tile([128, NKT * 130], BF16, tag="vx")
                    nc.sync.dma_start(
                        out=vx_h[:].rearrange("p (t n) -> p t n", t=NKT),
                        in_=vx[h].rearrange("(t p) n -> p t n", p=128))

                    pt = ptp.tile([128, NKT * ROWS], BF16, tag="pt")
                    for ktg in range(NKT // 4):
                        ps = sp.tile([128, 4 * ROWS], F32, tag="st")
                        for j in range(4):
                            kt = 4 * ktg + j
                            nc.tensor.matmul(
                                ps[:, ROWS * j: ROWS * (j + 1)],
                                lhsT=kT_h[:, 128 * kt: 128 * (kt + 1)],
                                rhs=qT_sb[:, h * ROWS: (h + 1) * ROWS],
                                start=True, stop=True)
                        nc.scalar.activation(
                            pt[:, 4 * ROWS * ktg: 4 * ROWS * (ktg + 1)],
                            ps[:], ACTF.Exp)
                    # mask-multiply (whole strip)
                    nc.vector.scalar_tensor_tensor(
                        out=pt[:], in0=pt[:], scalar=1.0, in1=mT[:],
                        op0=ALU.mult, op1=ALU.mult)

                    for cc in range(2):
                        qoff = 128 * cc
                        po = avp.tile([128, 130], F32, tag="av")
                        for kt in range(NKT):
                            nc.tensor.matmul(
                                po[:],
                                lhsT=pt[:, ROWS * kt + qoff:
                                        ROWS * kt + qoff + 128],
                                rhs=vx_h[:, 130 * kt: 130 * (kt + 1)],
                                start=(kt == 0), stop=(kt == NKT - 1))
                        nc.vector.tensor_copy(zbuf[cc][:, h:h + 1],
                                              po[:, 128:129])
                        rz = smallp.tile([128, 1], F32, tag="rz")
                        nc.vector.reciprocal(rz[:], po[:, 128:129])
                        o_sb = osb.tile([128, 128], F32, tag="o")
                        nc.scalar.activation(o_sb[:], po[:, 0:128],
                                             ACTF.Copy, scale=rz[:])
                        nc.sync.dma_start(
                            out=o_out[h, qoff:qoff + 128, :], in_=o_sb[:])

            # lnZ per chunk/head
            for cc in range(2):
                nc.scalar.activation(nlnz[cc][:], zbuf[cc][:], ACTF.Ln)
                nc.vector.tensor_scalar_mul(out=nlnz[cc][:], in0=nlnz[cc][:],
                                            scalar1=-1.0)

            # ---------------- phase D: mad + loss, chunk-major
            with tc.tile_pool(name="dk", bufs=2) as dkp, \
                 tc.tile_pool(name="madps", bufs=1, space="PSUM") as madp, \
                 tc.tile_pool(name="qkq", bufs=2, space="PSUM") as qkp, \
                 tc.tile_pool(name="eq", bufs=2) as eqp, \
                 tc.tile_pool(name="losstmp", bufs=1) as ltp:
                for cc in range(2):
                    qoff = 128 * cc
                    mad = madp.tile([128, S], F32, tag="mad")
                    for h in range(H):
                        kT_h2 = dkp.tile([128, S], BF16, tag="kT2")
                        nc.sync.dma_start(out=kT_h2[:], in_=kT[h, :, :])
                        for ks in range(S // 512):
                            p2 = qkp.tile([128, 512], F32, tag="p2")
                            nc.tensor.matmul(
                                p2[:],
                                lhsT=qT_sb[:, h * ROWS + qoff:
                                           h * ROWS + qoff + 128],
                                rhs=kT_h2[:, 512 * ks: 512 * (ks + 1)],
                                start=True, stop=True)
                            eq = eqp.tile([128, 512], BF16, tag="eq")
                            nc.scalar.activation(eq[:], p2[:], ACTF.Exp,
                                                 bias=nlnz[cc][:, h:h + 1])
                            nc.tensor.matmul(
                                mad[:, 512 * ks: 512 * (ks + 1)],
                                lhsT=id_sb[:], rhs=eq[:],
                                start=(h == 0), stop=(h == H - 1))

                    madE = ltp.tile([128, S], F32, tag="madE")
                    nc.vector.tensor_scalar(
                        out=madE[:], in0=mad[:], scalar1=1.0 / 16.0,
                        scalar2=EPS, op0=ALU.mult, op1=ALU.add)
                    lnT = ltp.tile([128, S], F32, tag="lnT")
                    nc.scalar.activation(lnT[:], madE[:], ACTF.Ln)
                    madm = ltp.tile([128, S], F32, tag="madm")
                    nc.vector.scalar_tensor_tensor(
                        out=madm[:], in0=madE[:], scalar=1.0,
                        in1=m_bf[cc][:], op0=ALU.mult, op1=ALU.mult)
                    junk2 = ltp.tile([128, S], BF16, tag="junk2")
                    accA = smallp.tile([128, 1], F32, tag="accA")
                    nc.vector.scalar_tensor_tensor(
                        out=junk2[:], in0=lnT[:], scalar=0.0,
                        in1=madm[:], op0=ALU.add, op1=ALU.mult,
                        accum_out=accA[:])

                    et = ltp.tile([128, S], BF16, tag="et")
                    nc.scalar.activation(et[:], isc[cc][:], ACTF.Exp,
                                         bias=negmx[cc][:])
                    etm = ltp.tile([128, S], BF16, tag="etm")
                    zp = smallp.tile([128, 1], F32, tag="zp")
                    nc.vector.scalar_tensor_tensor(
                        out=etm[:], in0=et[:], scalar=1.0,
                        in1=m_bf[cc][:], op0=ALU.mult, op1=ALU.mult,
                        accum_out=zp[:])
                    rzp = smallp.tile([128, 1], F32, tag="rzp")
                    nc.vector.reciprocal(rzp[:], zp[:])
                    lnP = ltp.tile([128, S], F32, tag="lnP")
                    nc.scalar.activation(lnP[:], etm[:], ACTF.Ln,
                                         bias=epsb[:], scale=rzp[:])
                    accB = smallp.tile([128, 1], F32, tag="accB")
                    nc.vector.scalar_tensor_tensor(
                        out=junk2[:], in0=lnP[:], scalar=0.0,
                        in1=madm[:], op0=ALU.add, op1=ALU.mult,
                        accum_out=accB[:])
                    klv = smallp.tile([128, 1], F32, tag="klv")
                    nc.vector.tensor_tensor(out=klv[:], in0=accA[:],
                                            in1=accB[:], op=ALU.subtract)
                    nc.sync.dma_start(out=kl_out[qoff:qoff + 128, :],
                                      in_=klv[:])
    return nc


def _legalize_waits(nc, max_waits=1):
    """walrus in this env rejects >2 sem-waits per instruction: move the
    excess onto same-engine nops inserted right before the instruction."""
    ctr = [0]
    f = nc.m.functions[0]
    for blk in f.blocks:
        newl = []
        for ins in blk.instructions:
            si = ins.sync_info
            waits = list(si.on_wait) if si and si.on_wait else []
            if len(waits) > max_waits:
                excess = waits[:-max_waits]
                for i in range(0, len(excess), max_waits):
                    nop = mybir.InstNoOp(
                        name=f"waitsplit-{ctr[0]}", ins=[], outs=[])
                    ctr[0] += 1
                    nop.engine = ins.engine
                    nop.sync_info = mybir.SyncInfo(
                        on_wait=excess[i:i + max_waits], on_update=[])
                    newl.append(nop)
                ins.sync_info = mybir.SyncInfo(
                    on_wait=waits[-max_waits:],
                    on_update=list(si.on_update) if si.on_update else [])
            newl.append(ins)
        try:
            blk.instructions = newl
        except AttributeError:
            del blk.instructions[:]
            blk.instructions.extend(newl)


_PROGRAM = None


def _get_program():
    global _PROGRAM
    if _PROGRAM is None:
        _apply_patch()
        _PROGRAM = build_program()
        _legalize_waits(_PROGRAM)
    return _PROGRAM


def _prep_inputs(q, k, v, q_indexer, k_indexer, weights):
    bf = ml_dtypes.bfloat16
    scale = 1.0 / np.sqrt(D)
    q = np.asarray(q, np.float32)[0]          # [H,S,D]
    k = np.asarray(k, np.float32)[0]
    v = np.asarray(v, np.float32)[0]
    qi = np.asarray(q_indexer, np.float32)[0]  # [S,HI,DI]
    ki = np.asarray(k_indexer, np.float32)[0, :, 0]  # [S,DI]
    w = np.asarray(weights, np.float32)[0]    # [S,HI]

    kT = np.ascontiguousarray(k.transpose(0, 2, 1)).astype(bf)   # [H,D,S]
    vxf = np.zeros((H, S, 130), np.float32)
    vxf[:, :, :128] = v
    vxf[:, :, 128] = 1.0
    vx = vxf.astype(bf)
    kiT = np.ascontiguousarray(ki.T)                             # [DI,S]
    qiw = qi * w[:, :, None]                                     # [S,HI,DI]
    ident = np.eye(128, dtype=np.float32).astype(bf)

    jj = np.arange(S, dtype=np.int64)
    in_maps = []
    for c in range(N_CORES):
        rows = slice(ROWS * c, ROWS * (c + 1))
        qTc = np.ascontiguousarray(
            (q[:, rows, :] * scale).transpose(0, 2, 1)).astype(bf)  # [H,D,R]
        qiTc = np.ascontiguousarray(
            qiw[rows].transpose(1, 2, 0)).astype(np.float32)        # [HI,DI,R]
        qpos = np.arange(ROWS * c, ROWS * (c + 1))[:, None]
        cmc = np.where(jj[None, :] <= qpos, 0.0, NEG_BIG).astype(np.float32)
        in_maps.append({
            "qT": qTc, "qiT": qiTc, "kT": kT, "vx": vx, "kiT": kiT,
            "cm": cmc, "ident": ident,
        })
    return in_maps


def kernel(q, k, v, q_indexer, k_indexer, weights, index_topk, end_pos):
    assert int(index_topk) == K_TOP and int(end_pos) == S
    nc = _get_program()
    in_maps = _prep_inputs(q, k, v, q_indexer, k_indexer, weights)
    res = run_bass_kernel_spmd(nc, in_maps, list(range(N_CORES)))

    out = np.empty((B, H, S, D), np.float32)
    kl_rows = np.empty((S,), np.float64)
    for c in range(N_CORES):
        rows = slice(ROWS * c, ROWS * (c + 1))
        out[0, :, rows, :] = res.results[c]["o"]
        kl_rows[rows.start:rows.stop] = res.results[c]["kl"][:, 0]
    loss = np.float32(kl_rows.mean())
    return loss, out
